# revision 21
# baseline (speedup 1.0000x reference)
"""AttentionDecoder (Bahdanau attention + 2-layer LSTM, T=64 steps) on 8 TRN2
NeuronCores. Data-parallel over batch: each core owns 8 of the 64 batch rows;
all weights replicated; the T-step recurrence runs fully on-chip per core with
no collectives.

v2: PE col-tiling (tile_position) runs 4 matmuls concurrently in the 128x128
array -- gates map gate g -> column group g (out partitions 32g..32g+7),
attention scores/context map batch b -> group b%4 with zero-padded stationary
columns for the second batch of each group. LSTM gate activations collapse to
ONE tanh over [128,512] (i,f,o weight blocks pre-scaled 0.5 on the host so
sigmoid(x) = 0.5*tanh(0.5x)+0.5 shares the tanh) + one per-partition affine.

Self-contained: hardcodes B=64, S=512, H=512, T=64, 8 cores.
"""
import sys
sys.path.insert(0, '/opt/trn_rl_repo')

import numpy as np
import ml_dtypes

import concourse.bass as bass
from concourse import bacc
import concourse.mybir as mybir
from concourse.tile import TileContext
from concourse.bass_utils import run_bass_kernel_spmd
from concourse.masks import make_identity

BF16 = mybir.dt.bfloat16
F32 = mybir.dt.float32
AF = mybir.ActivationFunctionType
OP = mybir.AluOpType
nbf16 = ml_dtypes.bfloat16

N_CORES = 8
B, S, H, T = 64, 512, 512, 64
BL = B // N_CORES          # 8 local batch rows
G4 = 4 * H                 # 2048 gate rows
P = 128

_cache = {}
_last_result = None


def build(has_b0, has_b1, has_ba):
    nc = bacc.Bacc("TRN2", target_bir_lowering=False, debug=False,
                   num_devices=N_CORES)

    dp = lambda name, shape, dt: nc.dram_tensor(
        name, shape, dt, kind="ExternalInput").ap()

    enc_s = dp("enc_s", [P, BL * 4 * S], BF16)      # (p,(b,st,h)) s on partitions
    enc_t = dp("enc_t", [BL * 4, P, S], BF16)       # ((b,et),p,s) h on partitions
    wih0T = dp("wih0T", [P, 8 * G4], BF16)          # (p,(kt8,g*H+n)) ifo blocks x0.5
    whh0T = dp("whh0T", [P, 4 * G4], BF16)
    wih1T = dp("wih1T", [P, 4 * G4], BF16)
    whh1T = dp("whh1T", [P, 4 * G4], BF16)
    wahT = dp("wahT", [P, 4 * H], BF16)             # (p,(kt4,ho))
    waeT = dp("waeT", [P, 4 * H], BF16)             # (p,(et4,h))
    wfbc = dp("wfbc", [BL, H], BF16)                # Wf row replicated 8x
    vz = dp("vz", [P, 4 * 33], BF16)                # per ht: [v,0*31,v]
    h00 = dp("h00", [BL, H], F32)
    c00 = dp("c00", [BL, H], F32)
    h01 = dp("h01", [BL, H], F32)
    c01 = dp("c01", [BL, H], F32)
    if has_ba:
        baRow = dp("baRow", [1, H], BF16)
    if has_b0:
        b0Row = dp("b0Row", [1, G4], BF16)          # ifo blocks x0.5
    if has_b1:
        b1Row = dp("b1Row", [1, G4], BF16)
    out = nc.dram_tensor("out", [BL, T], F32, kind="ExternalOutput").ap()

    with TileContext(nc) as tc:
        with (
            tc.tile_pool(name="const", bufs=1) as cp,
            tc.tile_pool(name="etile", bufs=2) as epool,
            tc.tile_pool(name="ring", bufs=2) as ring,
            tc.tile_pool(name="wk", bufs=1) as wk,
            tc.tile_pool(name="wk2", bufs=2) as wk2,
            tc.tile_pool(name="ht", bufs=2) as htp,
            tc.tile_pool(name="psG", bufs=1, space="PSUM") as psG,
            tc.tile_pool(name="psA", bufs=2, space="PSUM") as psA,
            tc.tile_pool(name="psQ", bufs=1, space="PSUM") as psQ,
            tc.tile_pool(name="psT", bufs=1, space="PSUM") as psT,
        ):
            # ---------------- resident SBUF ----------------
            enc_sb = cp.tile([P, BL * 4 * S], BF16)
            nc.sync.dma_start(enc_sb[:], enc_s)
            wih0_sb = cp.tile([P, 8 * G4], BF16)
            nc.sync.dma_start(wih0_sb[:], wih0T)
            whh0_sb = cp.tile([P, 4 * G4], BF16)
            nc.sync.dma_start(whh0_sb[:], whh0T)
            wih1_sb = cp.tile([P, 4 * G4], BF16)
            nc.sync.dma_start(wih1_sb[:], wih1T)
            whh1_sb = cp.tile([P, 4 * G4], BF16)
            nc.sync.dma_start(whh1_sb[:], whh1T)
            wah_sb = cp.tile([P, 4 * H], BF16)
            nc.sync.dma_start(wah_sb[:], wahT)
            wf_sb = cp.tile([BL, H], BF16)
            nc.sync.dma_start(wf_sb[:], wfbc)
            vz_sb = cp.tile([P, 4 * 33], BF16)
            nc.sync.dma_start(vz_sb[:], vz)
            if has_ba:
                ba_sb = cp.tile([1, H], BF16)
                nc.sync.dma_start(ba_sb[:], baRow)
            if has_b0:
                b0_sb = cp.tile([1, G4], BF16)
                nc.sync.dma_start(b0_sb[:], b0Row)
            if has_b1:
                b1_sb = cp.tile([1, G4], BF16)
                nc.sync.dma_start(b1_sb[:], b1Row)

            ident = cp.tile([P, P], BF16)
            make_identity(nc, ident[:])
            ones18 = cp.tile([1, BL], BF16)
            nc.vector.memset(ones18[:], 1.0)
            # zero-interleaved wT storage: per (st,grp) 33 cols [w_p0, 0*31, w_p1]
            wTz = cp.tile([P, 16 * 33], BF16)
            nc.vector.memset(wTz[:], 0.0)
            ep_sb = cp.tile([P, BL * 4 * S], BF16)   # (p,(ht,b,s))
            y_acc = cp.tile([BL, T], F32)
            yprod = cp.tile([BL, H], F32)

            # ---------------- setup: ep = enc @ Wa_e.T (transposed layout) ---
            wae_sb = ring.tile([P, 4 * H], BF16, tag="wae")
            nc.sync.dma_start(wae_sb[:], waeT)
            for b in range(BL):
                eps = [psA.tile([P, S], F32, tag="psA", name=f"eps{_i}")
                       for _i in range(2)] + \
                      [psG.tile([P, S], F32, tag=f"psg{_i - 2}",
                                name=f"eps{_i}")
                       for _i in range(2, 4)]
                for et in range(4):
                    et_sb = ring.tile([P, S], BF16, tag="encT")
                    nc.sync.dma_start(et_sb[:], enc_t[b * 4 + et])
                    for ht in range(4):
                        nc.tensor.matmul(
                            eps[ht][:],
                            wae_sb[:, et * H + ht * P: et * H + (ht + 1) * P],
                            et_sb[:],
                            start=(et == 0), stop=(et == 3))
                for ht in range(4):
                    nc.scalar.copy(
                        ep_sb[:, (ht * BL + b) * S:(ht * BL + b + 1) * S],
                        eps[ht][:])

            # ---------------- setup: states ----------------
            # c states live at partition rows 32-39 (aligned with gate f)
            c0_f = cp.tile([P, H], F32)
            nc.sync.dma_start(c0_f[32:40, :], c00)
            c1_f = cp.tile([P, H], F32)
            nc.sync.dma_start(c1_f[32:40, :], c01)
            h0_f = wk.tile([BL, H], F32, tag="h0f")
            nc.sync.dma_start(h0_f[:], h00)
            h1_f = wk.tile([BL, H], F32, tag="h1f")
            nc.sync.dma_start(h1_f[:], h01)
            h0_bf = wk.tile([BL, H], BF16, tag="h0bf")
            nc.vector.tensor_copy(h0_bf[:], h0_f[:])
            h1_bf = wk.tile([BL, H], BF16, tag="h1bf")
            nc.vector.tensor_copy(h1_bf[:], h1_f[:])

            def transpose_8(src_bf, tag):
                """[8,512] bf16 (rows 0-7) -> [128,32] col 8j+b."""
                ps = psT.tile([P, 32], BF16, tag="psT")
                for j in range(4):
                    nc.tensor.transpose(ps[:, 8 * j:8 * j + 8],
                                        src_bf[0:8, 128 * j:128 * (j + 1)],
                                        ident[:8, :8])
                dst = htp.tile([P, 32], BF16, tag=tag)
                nc.scalar.copy(dst[:], ps[:])     # ACT idle here; spare DVE
                return dst

            h0T = transpose_8(h0_bf, "h0T")
            h1T = transpose_8(h1_bf, "h1T")

            def compute_qba(h1T_):
                """q = h1 @ Wa_h.T (+ba) -> [128,32] f32, col 8j+b."""
                q_ps = psQ.tile([BL, H], F32, tag="psQ")
                n_mm = 4 + (1 if has_ba else 0)
                for kt in range(4):
                    nc.tensor.matmul(q_ps[:], h1T_[:, 8 * kt:8 * kt + 8],
                                     wah_sb[:, kt * H:(kt + 1) * H],
                                     start=(kt == 0),
                                     stop=(kt == n_mm - 1))
                if has_ba:
                    nc.tensor.matmul(q_ps[:], ones18[:], ba_sb[:],
                                     start=False, stop=True)
                q_bf = wk.tile([BL, H], BF16, tag="q_bf")
                nc.scalar.copy(q_bf[:], q_ps[:])
                ps = psT.tile([P, 32], BF16, tag="psT")
                for j in range(4):
                    nc.tensor.transpose(ps[:, 8 * j:8 * j + 8],
                                        q_bf[0:8, 128 * j:128 * (j + 1)],
                                        ident[:8, :8])
                qdst = wk2.tile([P, 32], F32, tag="qbaT")
                nc.scalar.copy(qdst[:], ps[:])
                return qdst

            qbaT = compute_qba(h1T)

            def emit_g1_h(g1_, h1T_):
                for kt in range(4):
                    for g in range(4):
                        nc.tensor.matmul(
                            g1_[32 * g:32 * g + 8, :],
                            h1T_[:, 8 * kt:8 * kt + 8],
                            whh1_sb[:, kt * G4 + g * H:kt * G4 + (g + 1) * H],
                            start=(kt == 0), stop=False,
                            tile_position=(0, 32 * g))
                if has_b1:
                    for g in range(4):
                        nc.tensor.matmul(
                            g1_[32 * g:32 * g + 8, :], ones18[:],
                            b1_sb[:, g * H:(g + 1) * H],
                            start=False, stop=False,
                            tile_position=(0, 32 * g))

            def emit_g0_hdec(g0_, h0T_, h1T_):
                for kt in range(4):
                    for g in range(4):
                        nc.tensor.matmul(
                            g0_[32 * g:32 * g + 8, :],
                            h0T_[:, 8 * kt:8 * kt + 8],
                            whh0_sb[:, kt * G4 + g * H:kt * G4 + (g + 1) * H],
                            start=(kt == 0), stop=False,
                            tile_position=(0, 32 * g))
                if h1T_ is None:   # step 0: dec_in = 0
                    return
                for kt in range(4):
                    for g in range(4):
                        nc.tensor.matmul(
                            g0_[32 * g:32 * g + 8, :],
                            h1T_[:, 8 * kt:8 * kt + 8],
                            wih0_sb[:, kt * G4 + g * H:kt * G4 + (g + 1) * H],
                            start=False, stop=False,
                            tile_position=(0, 32 * g))
                if has_b0:
                    for g in range(4):
                        nc.tensor.matmul(
                            g0_[32 * g:32 * g + 8, :], ones18[:],
                            b0_sb[:, g * H:(g + 1) * H],
                            start=False, stop=False,
                            tile_position=(0, 32 * g))

            def lstm_elementwise(gps, c_f, sfx):
                """gps [128,512] psum: gate g rows 32g..32g+7 (i,f,g,o order,
                i/f/o pre-scaled 0.5 so tanh stands in for sigmoid). c_f rows
                32-39. Output h' = 2h (rows 0-7); downstream weights are
                pre-halved on the host to absorb the 2x."""
                t_all = wk.tile([P, H], BF16, tag="t_all")
                nc.scalar.activation(t_all[:], gps[:], AF.Tanh)
                # tg2 = 0.5*tanh(g)
                tg2 = wk.tile([P, H], BF16, tag="tg2")
                nc.vector.tensor_scalar(tg2[0:8, :], t_all[64:72, :],
                                        0.5, None, op0=OP.mult)
                # B = (tanh(i)+1)*tg2 = sigmoid(x_i)*tanh(x_g)
                bt = wk.tile([P, H], BF16, tag="bt")
                nc.vector.scalar_tensor_tensor(
                    bt[32:40, :], t_all[0:8, :], 1.0, tg2[0:8, :],
                    op0=OP.add, op1=OP.mult)
                # A = (tanh(f)+1)*c = 2*sigmoid(x_f)*c
                at = wk.tile([P, H], F32, tag="at")
                nc.vector.scalar_tensor_tensor(
                    at[32:40, :], t_all[32:40, :], 1.0, c_f[32:40, :],
                    op0=OP.add, op1=OP.mult)
                # c' = 0.5*A + B
                nc.vector.scalar_tensor_tensor(
                    c_f[32:40, :], at[32:40, :], 0.5, bt[32:40, :],
                    op0=OP.mult, op1=OP.add)
                tc2 = wk.tile([P, H], BF16, tag="tc2")
                nc.scalar.activation(tc2[96:104, :], c_f[32:40, :], AF.Tanh)
                # h' = (tanh(o)+1)*tanh(c) = 2h
                h_bf = wk2.tile([BL, H], BF16, tag="h_bf" + sfx)
                nc.vector.scalar_tensor_tensor(
                    h_bf[:], t_all[96:104, :], 1.0, tc2[96:104, :],
                    op0=OP.add, op1=OP.mult)
                return h_bf

            g0 = psG.tile([P, H], F32, tag="psg0")
            g1 = psG.tile([P, H], F32, tag="psg1")
            emit_g0_hdec(g0, h0T, None)

            # ---------------- the recurrence ----------------
            for t in range(T):
                # ---- early gates1 h-part: PE busy while DVE/ACT build e ----
                emit_g1_h(g1, h1T)

                # ---- energy + scores, pipelined per h-tile ----
                scs = psA.tile([P, S], F32, tag="psA", name="scs")
                for ht in range(4):
                    for par in range(2):
                        e_t = epool.tile([P, 4 * S], BF16, tag="e_t")
                        for i4 in range(4):
                            b = par * 4 + i4
                            nc.vector.tensor_scalar(
                                e_t[:, i4 * S:(i4 + 1) * S],
                                ep_sb[:, (ht * BL + b) * S:
                                      (ht * BL + b + 1) * S],
                                qbaT[:, 8 * ht + b:8 * ht + b + 1], None,
                                op0=OP.add)
                        nc.scalar.activation(e_t[:], e_t[:], AF.Tanh)
                        for grp in range(4):
                            if par == 0:
                                lhsT = vz_sb[:, 33 * ht:33 * ht + 32]
                                o = scs[32 * grp:32 * grp + 32, :]
                            else:
                                lhsT = vz_sb[:, 33 * ht + 31:33 * ht + 33]
                                o = scs[32 * grp:32 * grp + 2, :]
                            nc.tensor.matmul(
                                o, lhsT, e_t[:, grp * S:(grp + 1) * S],
                                start=(ht == 0 and par == 0),
                                stop=(ht == 3 and par == 1),
                                tile_position=(0, 32 * grp))

                # ---- softmax (unnormalized) ----
                w_sb = wk2.tile([P, S], BF16, tag="w_sb")
                zsum = wk.tile([P, 1], F32, tag="zsum")
                nc.scalar.activation(w_sb[:], scs[:], AF.Exp,
                                     accum_out=zsum[:])
                zrec = wk.tile([P, 1], F32, tag="zrec")
                nc.vector.reciprocal(zrec[:], zsum[:])

                # ---- w transpose -> zero-interleaved stationary ----
                trp = psT.tile([P, 4 * P], BF16, tag="trpw")
                for st in range(4):
                    nc.tensor.transpose(trp[:, P * st:P * st + 98],
                                        w_sb[0:98, P * st:P * (st + 1)],
                                        ident[:98, :98])
                trp_v = trp[:].rearrange("p (st g2 r) -> p st g2 r",
                                         st=4, g2=4, r=32)
                wTz_v = wTz[:].rearrange("p (sg c) -> p sg c", c=33)
                nc.scalar.copy(wTz_v[:, :, 0:1],
                               trp_v[:, :, :, 0:1].rearrange(
                                   "p st g2 r -> p (st g2) r"))
                nc.vector.tensor_copy(wTz_v[:, :, 32:33],
                                      trp_v[:, :, :, 1:2].rearrange(
                                          "p st g2 r -> p (st g2) r"))

                # ---- context ----
                ctxps = psA.tile([P, H], F32, tag="psA", name="ctxps")
                for st in range(4):
                    for grp in range(4):
                        c0 = 33 * (4 * st + grp)
                        for par in range(2):
                            b = par * 4 + grp
                            if par == 0:
                                lhsT = wTz[:, c0:c0 + 32]
                                o = ctxps[32 * grp:32 * grp + 32, :]
                            else:
                                lhsT = wTz[:, c0 + 31:c0 + 33]
                                o = ctxps[32 * grp:32 * grp + 2, :]
                            nc.tensor.matmul(
                                o, lhsT,
                                enc_sb[:, (b * 4 + st) * S:(b * 4 + st + 1) * S],
                                start=(st == 0 and par == 0),
                                stop=(st == 3 and par == 1),
                                tile_position=(0, 32 * grp))

                # ---- ctx scale + transpose ----
                ctx_bf = wk2.tile([P, H], BF16, tag="ctx_bf")
                nc.vector.tensor_scalar(ctx_bf[:], ctxps[:],
                                        zrec[:, 0:1], None, op0=OP.mult)
                trc = psT.tile([P, 4 * P], BF16, tag="trpc")
                for j in range(4):
                    nc.tensor.transpose(trc[:, P * j:P * j + 98],
                                        ctx_bf[0:98, P * j:P * (j + 1)],
                                        ident[:98, :98])
                ctxT = wk2.tile([P, 32], BF16, tag="ctxT")
                trc_v = trc[:].rearrange("p (j g2 r) -> p j g2 r",
                                         j=4, g2=4, r=32)
                ctxT_v = ctxT[:].rearrange("p (j par g2) -> p j g2 par",
                                           j=4, par=2, g2=4)
                nc.vector.tensor_copy(ctxT_v[:], trc_v[:, :, :, 0:2])

                # ---- finish gates0: ctx part (koffset 4 in wih0) ----
                for kt in range(4):
                    for g in range(4):
                        nc.tensor.matmul(
                            g0[32 * g:32 * g + 8, :],
                            ctxT[:, 8 * kt:8 * kt + 8],
                            wih0_sb[:, (4 + kt) * G4 + g * H:
                                    (4 + kt) * G4 + (g + 1) * H],
                            start=False, stop=(kt == 3),
                            tile_position=(0, 32 * g))

                # ---- LSTM layer 0 ----
                h0n_bf = lstm_elementwise(g0, c0_f, "0")
                h0T = transpose_8(h0n_bf, "h0T")

                # ---- LSTM layer 1: x-part then elementwise ----
                for kt in range(4):
                    for g in range(4):
                        nc.tensor.matmul(
                            g1[32 * g:32 * g + 8, :],
                            h0T[:, 8 * kt:8 * kt + 8],
                            wih1_sb[:, kt * G4 + g * H:kt * G4 + (g + 1) * H],
                            start=False, stop=(kt == 3),
                            tile_position=(0, 32 * g))
                h1n_bf = lstm_elementwise(g1, c1_f, "1")
                h1T = transpose_8(h1n_bf, "h1T")

                if t + 1 < T:
                    # q first: qbaT feeds next step's energy (critical path),
                    # then next step's gates0 h/dec parts keep PE warm
                    qbaT = compute_qba(h1T)
                    g0 = psG.tile([P, H], F32, tag="psg0")
                    g1 = psG.tile([P, H], F32, tag="psg1")
                    emit_g0_hdec(g0, h0T, h1T)

                # ---- y = h1n @ Wf.T (DVE mult + reduce, no PSUM) ----
                nc.vector.tensor_mul(yprod[:], h1n_bf[:], wf_sb[:])
                nc.vector.tensor_reduce(y_acc[0:BL, t:t + 1], yprod[:],
                                        axis=mybir.AxisListType.X,
                                        op=OP.add)

            nc.sync.dma_start(out[:], y_acc[:])

    nc.compile()
    return nc


def _marshal(inputs):
    """Host-side shard + relayout. Returns (in_maps, flags)."""
    f32 = np.float32
    enc = np.asarray(inputs["encoder_outputs"], f32)
    Wa_h = np.asarray(inputs["Wa_h"], f32)
    Wa_e = np.asarray(inputs["Wa_e"], f32)
    ba = np.asarray(inputs["ba"], f32)
    v = np.asarray(inputs["v"], f32)
    W_ih_0 = np.asarray(inputs["W_ih_0"], f32).copy()
    W_hh_0 = np.asarray(inputs["W_hh_0"], f32).copy()
    b_0 = np.asarray(inputs["b_0"], f32).copy()
    W_ih_1 = np.asarray(inputs["W_ih_1"], f32).copy()
    W_hh_1 = np.asarray(inputs["W_hh_1"], f32).copy()
    b_1 = np.asarray(inputs["b_1"], f32).copy()
    Wf = np.asarray(inputs["Wf"], f32)

    has_ba = bool(np.any(ba != 0))
    has_b0 = bool(np.any(b_0 != 0))
    has_b1 = bool(np.any(b_1 != 0))

    # h' = 2h convention: the kernel's LSTM emits doubled hidden states
    # ((tanh(o)+1)*tanh(c) = 2h), so every weight that consumes an h is
    # pre-halved, and the initial h states are doubled.
    Wa_h = Wa_h * 0.5
    Wf = Wf * 0.5
    W_hh_0 *= 0.5                      # consumes h0'
    W_hh_1 *= 0.5                      # consumes h1'
    W_ih_1 *= 0.5                      # consumes h0n' (layer-1 input)
    W_ih_0[:, 0:H] *= 0.5              # dec_in = h1' block (ctx block as-is)
    h0_0 = np.asarray(inputs["h0_0"], f32) * 2.0
    h0_1 = np.asarray(inputs["h0_1"], f32) * 2.0

    # pre-scale i,f,o gate blocks (rows 0:H, H:2H, 3H:4H) by 0.5 so
    # sigmoid(x) = 0.5*tanh(0.5x) + 0.5 shares one tanh with the cell gate
    for W in (W_ih_0, W_hh_0, W_ih_1, W_hh_1):
        W[0:2 * H] *= 0.5
        W[3 * H:4 * H] *= 0.5
    for bb in (b_0, b_1):
        bb[0:2 * H] *= 0.5
        bb[3 * H:4 * H] *= 0.5

    def to_kxn(W, n_kt):  # W [N, K] -> [128, n_kt*N] bf16 layout (p,(kt,n))
        Wt = W.T.astype(nbf16)                       # [K, N]
        return np.ascontiguousarray(
            Wt.reshape(n_kt, P, W.shape[0]).transpose(1, 0, 2).reshape(P, -1))

    wih0T = to_kxn(W_ih_0, 8)
    whh0T = to_kxn(W_hh_0, 4)
    wih1T = to_kxn(W_ih_1, 4)
    whh1T = to_kxn(W_hh_1, 4)
    wahT = to_kxn(Wa_h, 4)
    waeT = to_kxn(Wa_e, 4)
    wfbc = np.ascontiguousarray(
        np.broadcast_to(Wf.reshape(1, H), (BL, H)).astype(nbf16))

    vzm = np.zeros((P, 4 * 33), nbf16)
    for ht in range(4):
        vzm[:, 33 * ht] = v[ht * P:(ht + 1) * P].astype(nbf16)
        vzm[:, 33 * ht + 32] = v[ht * P:(ht + 1) * P].astype(nbf16)

    shared = dict(wih0T=wih0T, whh0T=whh0T, wih1T=wih1T, whh1T=whh1T,
                  wahT=wahT, waeT=waeT, wfbc=wfbc, vz=vzm)
    if has_ba:
        shared["baRow"] = ba.reshape(1, H).astype(nbf16)
    if has_b0:
        shared["b0Row"] = b_0.reshape(1, G4).astype(nbf16)
    if has_b1:
        shared["b1Row"] = b_1.reshape(1, G4).astype(nbf16)

    enc_bf = enc.astype(nbf16)
    in_maps = []
    for c in range(N_CORES):
        sl = slice(c * BL, (c + 1) * BL)
        eb = enc_bf[sl]                                   # [8, 512, 512]
        enc_s = np.ascontiguousarray(
            eb.reshape(BL, 4, P, H).transpose(2, 0, 1, 3).reshape(P, -1))
        enc_t = np.ascontiguousarray(
            eb.transpose(0, 2, 1).reshape(BL, 4, P, S).reshape(BL * 4, P, S))
        m = dict(shared)
        m.update(
            enc_s=enc_s, enc_t=enc_t,
            h00=np.ascontiguousarray(h0_0[sl]),
            c00=np.ascontiguousarray(np.asarray(inputs["c0_0"], f32)[sl]),
            h01=np.ascontiguousarray(h0_1[sl]),
            c01=np.ascontiguousarray(np.asarray(inputs["c0_1"], f32)[sl]),
        )
        in_maps.append(m)
    return in_maps, (has_b0, has_b1, has_ba)


def kernel(**inputs):
    global _last_result
    in_maps, flags = _marshal(inputs)
    if flags not in _cache:
        _cache[flags] = build(*flags)
    nc = _cache[flags]
    res = run_bass_kernel_spmd(nc, in_maps, core_ids=list(range(N_CORES)))
    _last_result = res
    ys = np.concatenate([np.asarray(res.results[i]["out"], np.float32)
                         for i in range(N_CORES)], axis=0)   # [64, 64]
    bf_ = np.asarray(inputs["bf"], np.float32).reshape(1, 1)
    y = ys + bf_
    return y.reshape(B, T, 1).astype(np.float32)


if __name__ == "__main__":
    rng = np.random.default_rng(0)
    fake = {
        "encoder_outputs": rng.normal(size=(B, S, H)).astype(np.float32),
        "h0_0": rng.normal(size=(B, H)).astype(np.float32),
        "c0_0": rng.normal(size=(B, H)).astype(np.float32),
        "h0_1": rng.normal(size=(B, H)).astype(np.float32),
        "c0_1": rng.normal(size=(B, H)).astype(np.float32),
        "Wa_h": (rng.normal(size=(H, H)) * 0.05).astype(np.float32),
        "Wa_e": (rng.normal(size=(H, H)) * 0.05).astype(np.float32),
        "ba": np.zeros(H, np.float32),
        "v": (rng.normal(size=H) * 0.05).astype(np.float32),
        "W_ih_0": (rng.normal(size=(G4, 2 * H)) * 0.05).astype(np.float32),
        "W_hh_0": (rng.normal(size=(G4, H)) * 0.05).astype(np.float32),
        "b_0": np.zeros(G4, np.float32),
        "W_ih_1": (rng.normal(size=(G4, H)) * 0.05).astype(np.float32),
        "W_hh_1": (rng.normal(size=(G4, H)) * 0.05).astype(np.float32),
        "b_1": np.zeros(G4, np.float32),
        "Wf": (rng.normal(size=(1, H)) * 0.05).astype(np.float32),
        "bf": np.zeros(1, np.float32),
    }
    y = kernel(**fake)
    print("kernel output", y.shape, y.dtype, float(np.abs(y).max()))
